# revision 2
# baseline (speedup 1.0000x reference)
"""Trainium2 Bass kernel for single-head causal attention.

Problem: x [4, 4096, 1024], Wk/Wq/Wv [64, 1024] -> out [4, 4096, 64]
  k/q/v = x @ W^T;  out = softmax(causal(q k^T / 8)) @ v

Sharding (8 cores, one program): 2 cores per batch, striped (parity)
sequence-parallel over T. Core c handles batch c//2, query rows of parity
c%2 (rows h, h+2, ...). Each core computes K/V for both parities
(redundant with its pair partner, no collectives) and full attention for
its 2048 query rows. Scores are built transposed (S^T[k,q]) so the
exp'd probabilities feed the AV matmul directly as the moving operand;
V is augmented with a ones column so the softmax denominator accumulates
in the same PSUM tile. The host does the final divide + row scatter.

Causality at block granularity is a static triangle identical on every
core; the only per-core difference is the 128x128 diagonal-block mask
(inclusive vs strict), supplied as input data.
"""

import numpy as np

B, T, C, H = 4, 4096, 1024, 64
NCORES = 8
TL = T // 2          # local query rows per core
NB = TL // 128       # 16 local 128-row blocks
NQT = TL // 512      # 4 q-tiles of 512
NE = C // 128        # 8 contraction chunks
SCALE = 1.0 / np.sqrt(H)

_CACHE = {}


def _build_program():
    import concourse.bacc as bacc
    import concourse.tile as tile
    import concourse.mybir as mybir

    F32 = mybir.dt.float32
    F32R = mybir.dt.float32r
    EXP = mybir.ActivationFunctionType.Exp

    nc = bacc.Bacc("TRN2", target_bir_lowering=False, debug=False,
                   num_devices=NCORES)

    xt0_ap = nc.dram_tensor("xt0", [C, TL], F32, kind="ExternalInput").ap()
    xt1_ap = nc.dram_tensor("xt1", [C, TL], F32, kind="ExternalInput").ap()
    wk_ap = nc.dram_tensor("wk", [C, H], F32, kind="ExternalInput").ap()
    wq_ap = nc.dram_tensor("wq", [C, H], F32, kind="ExternalInput").ap()
    wv_ap = nc.dram_tensor("wv", [C, H], F32, kind="ExternalInput").ap()
    m0_ap = nc.dram_tensor("m0", [128, 128], F32, kind="ExternalInput").ap()
    m1_ap = nc.dram_tensor("m1", [128, 128], F32, kind="ExternalInput").ap()
    ident_ap = nc.dram_tensor("ident", [64, 64], F32, kind="ExternalInput").ap()
    ones_ap = nc.dram_tensor("ones", [128, 1], F32, kind="ExternalInput").ap()
    outT_ap = nc.dram_tensor("outT", [H + 1, TL], F32, kind="ExternalOutput").ap()

    with tile.TileContext(nc) as tc:
        with (
            tc.tile_pool(name="consts", bufs=1) as consts,
            tc.tile_pool(name="persist", bufs=1) as persist,
            tc.tile_pool(name="xin", bufs=3) as xin,
            tc.tile_pool(name="vt", bufs=2) as vtp,
            tc.tile_pool(name="pb", bufs=3) as pbp,
            tc.tile_pool(name="ob", bufs=2) as obp,
            tc.tile_pool(name="psA", bufs=2, space="PSUM") as psA,
            tc.tile_pool(name="psT", bufs=1, space="PSUM") as psT,
            tc.tile_pool(name="psS", bufs=2, space="PSUM") as psS,
            tc.tile_pool(name="psO", bufs=1, space="PSUM") as psO,
        ):
            # ---- constants ----
            wk_sb = consts.tile([128, NE, H], F32R)
            wq_sb = consts.tile([128, NE, H], F32R)
            wv_sb = consts.tile([128, NE, H], F32R)
            for dst, src in ((wk_sb, wk_ap), (wq_sb, wq_ap), (wv_sb, wv_ap)):
                nc.sync.dma_start(
                    out=dst[:],
                    in_=src.rearrange("(c p) h -> p c h", p=128).bitcast(F32R))
            m0_sb = consts.tile([128, 128], F32R)
            m1_sb = consts.tile([128, 128], F32R)
            ident = consts.tile([64, 64], F32R)
            ones_sb = consts.tile([128, 1], F32R)
            nc.sync.dma_start(out=m0_sb[:], in_=m0_ap[:].bitcast(F32R))
            nc.sync.dma_start(out=m1_sb[:], in_=m1_ap[:].bitcast(F32R))
            nc.sync.dma_start(out=ident[:], in_=ident_ap[:].bitcast(F32R))
            nc.sync.dma_start(out=ones_sb[:], in_=ones_ap[:].bitcast(F32R))

            # ---- persistent projections ----
            kT_sb = [persist.tile([64, TL], F32R, name=f"kT{p}", tag=f"kT{p}")
                     for p in (0, 1)]
            qT_sb = persist.tile([64, TL], F32R)
            v_sb = [persist.tile([128, NB, H + 1], F32R, name=f"v{p}", tag=f"v{p}")
                    for p in (0, 1)]

            # ---- projections ----
            for par, xt_ap in ((0, xt0_ap), (1, xt1_ap)):
                for t in range(NQT):
                    x_sb = xin.tile([128, NE, 512], F32R)
                    nc.sync.dma_start(
                        out=x_sb[:],
                        in_=xt_ap[:, t * 512:(t + 1) * 512]
                        .rearrange("(c p) m -> p c m", p=128).bitcast(F32R))

                    kps = psA.tile([64, 512], F32, tag="proj")
                    for e in range(NE):
                        nc.tensor.matmul(kps[:], wk_sb[:, e, :], x_sb[:, e, :],
                                         start=(e == 0), stop=(e == NE - 1))
                    nc.vector.tensor_copy(
                        kT_sb[par][:, t * 512:(t + 1) * 512], kps[:])

                    if par == 0:
                        qps = psA.tile([64, 512], F32, tag="proj")
                        for e in range(NE):
                            nc.tensor.matmul(qps[:], wq_sb[:, e, :],
                                             x_sb[:, e, :],
                                             start=(e == 0), stop=(e == NE - 1))
                        nc.vector.tensor_copy(
                            qT_sb[:, t * 512:(t + 1) * 512], qps[:])

                    vps = psA.tile([64, 512], F32, tag="proj")
                    for e in range(NE):
                        nc.tensor.matmul(vps[:], wv_sb[:, e, :], x_sb[:, e, :],
                                         start=(e == 0), stop=(e == NE - 1))
                    vT_sb = vtp.tile([64, 512], F32R)
                    nc.vector.tensor_copy(vT_sb[:], vps[:])
                    for u in range(4):
                        tps = psT.tile([128, 64], F32R)
                        nc.tensor.transpose(tps[:], vT_sb[:, u * 128:(u + 1) * 128],
                                            ident[:])
                        blk = 4 * t + u
                        nc.vector.tensor_copy(v_sb[par][:, blk, 0:H], tps[:])
                # denominator ones column
                for blk in range(NB):
                    nc.vector.tensor_copy(v_sb[par][:, blk, H:H + 1], ones_sb[:])

            # ---- attention ----
            for qt in range(NQT):
                o_ps = psO.tile([H + 1, 512], F32)
                nkb = 4 * qt + 4
                for s in (0, 1):
                    kT = kT_sb[s]
                    vv = v_sb[s]
                    msk = m0_sb if s == 0 else m1_sb
                    for g in range(0, nkb, 2):
                        kbs = [g, g + 1]
                        offs, widths = [], []
                        pos = 0
                        for kb in kbs:
                            qs = max(0, (kb - 4 * qt) * 128)
                            offs.append((pos, qs))
                            widths.append(512 - qs)
                            pos += 512 - qs
                        sg = psS.tile([128, 1024], F32, tag="scores")
                        for (pos0, qs), w, kb in zip(offs, widths, kbs):
                            nc.tensor.matmul(
                                sg[:, pos0:pos0 + w],
                                kT[:, kb * 128:(kb + 1) * 128],
                                qT_sb[:, qt * 512 + qs:(qt + 1) * 512],
                                start=True, stop=True)
                        p_sb = pbp.tile([128, 1024], F32R, tag="probs")
                        nc.scalar.activation(p_sb[:, 0:pos], sg[:, 0:pos], EXP,
                                             scale=float(SCALE))
                        for (pos0, qs), w, kb in zip(offs, widths, kbs):
                            if kb >= 4 * qt:
                                nc.vector.tensor_mul(
                                    p_sb[:, pos0:pos0 + 128],
                                    p_sb[:, pos0:pos0 + 128], msk[:])
                            nc.tensor.matmul(
                                o_ps[:, qs:512],
                                vv[:, kb, :],
                                p_sb[:, pos0:pos0 + w],
                                start=(s == 0 and kb == 0),
                                stop=(s == 1 and kb == nkb - 1))
                o_sb = obp.tile([H + 1, 512], F32)
                nc.vector.tensor_copy(o_sb[:], o_ps[:])
                nc.sync.dma_start(out=outT_ap[:, qt * 512:(qt + 1) * 512],
                                  in_=o_sb[:])

    nc.compile()
    return nc


def _get_program():
    if "nc" not in _CACHE:
        _CACHE["nc"] = _build_program()
    return _CACHE["nc"]


def kernel(x, Wk, Wq, Wv, i, embed_dim, head_size_sel, **_unused):
    from concourse import bass_utils

    x = np.asarray(x, dtype=np.float32)
    Wk = np.asarray(Wk, dtype=np.float32)
    Wq = np.asarray(Wq, dtype=np.float32)
    Wv = np.asarray(Wv, dtype=np.float32)

    nc = _get_program()

    idx = np.arange(128)
    m_incl = (idx[:, None] <= idx[None, :]).astype(np.float32)
    m_strict = (idx[:, None] < idx[None, :]).astype(np.float32)
    ident = np.eye(64, dtype=np.float32)
    ones = np.ones((128, 1), dtype=np.float32)
    wk_t = np.ascontiguousarray(Wk.T)   # [C, H]
    wq_t = np.ascontiguousarray(Wq.T)
    wv_t = np.ascontiguousarray(Wv.T)

    in_maps = []
    for c in range(NCORES):
        b, h = c // 2, c % 2
        in_maps.append({
            "xt0": np.ascontiguousarray(x[b, h::2, :].T),
            "xt1": np.ascontiguousarray(x[b, 1 - h::2, :].T),
            "wk": wk_t, "wq": wq_t, "wv": wv_t,
            "m0": m_incl,
            "m1": m_strict if h == 0 else m_incl,
            "ident": ident,
            "ones": ones,
        })

    res = bass_utils.run_bass_kernel_spmd(nc, in_maps,
                                          core_ids=list(range(NCORES)))
    _CACHE["last_result"] = res

    out = np.empty((B, T, H), dtype=np.float32)
    for c in range(NCORES):
        b, h = c // 2, c % 2
        outT = res.results[c]["outT"]
        num = outT[:H, :]          # [H, TL]
        den = outT[H, :]           # [TL]
        out[b, h::2, :] = (num / den[None, :]).T
    return out


# revision 3
# speedup vs baseline: 1.5313x; 1.5313x over previous
"""Trainium2 Bass kernel for single-head causal attention.

Problem: x [4, 4096, 1024], Wk/Wq/Wv [64, 1024] -> out [4, 4096, 64]
  k/q/v = x @ W^T;  out = softmax(causal(q k^T / 8)) @ v

Sharding (8 cores, one program): 2 cores per batch, striped (parity)
sequence-parallel over T. Core c handles batch c//2, query rows of parity
c%2 (rows h, h+2, ...). Each core computes K/V for both parities
(redundant with its pair partner) and full attention for its 2048 query
rows. Scores are built transposed (S^T[k,q]) so the exp'd probabilities
feed the AV matmul directly as the moving operand; V is augmented with a
ones column so the softmax denominator accumulates in the same PSUM
tile. The host does the final divide + row scatter.

Causality at block granularity is a static triangle identical on every
core; the only per-core difference is the 128x128 diagonal-block mask
(inclusive vs strict), supplied as input data.

Perf notes (measured on HW):
- matmuls with 64-partition operands stream at 2 cyc/col regardless of
  dtype; K^T/Q^T are therefore kept in 128-partition tiles with the
  bottom half zeroed, restoring 1 cyc/col for the scores matmul.
- fp16 operands: full-rate PE + FWL weight loads + half the DMA bytes.
- K and Q (and K and V for the other parity) projections are packed
  side-by-side in one 128-column stationary operand so one rhs stream
  produces both heads of output.
"""

import numpy as np

B, T, C, H = 4, 4096, 1024, 64
NCORES = 8
TL = T // 2          # local query rows per core
NB = TL // 128       # 16 local 128-row blocks
NQT = TL // 512      # 4 q-tiles of 512
NE = C // 128        # 8 contraction chunks
SCALE = 1.0 / np.sqrt(H)

_CACHE = {}


def _build_program():
    import concourse.bacc as bacc
    import concourse.tile as tile
    import concourse.mybir as mybir

    F32 = mybir.dt.float32
    F16 = mybir.dt.float16
    EXP = mybir.ActivationFunctionType.Exp

    nc = bacc.Bacc("TRN2", target_bir_lowering=False, debug=False,
                   num_devices=NCORES)

    xt0_ap = nc.dram_tensor("xt0", [C, TL], F16, kind="ExternalInput").ap()
    xt1_ap = nc.dram_tensor("xt1", [C, TL], F16, kind="ExternalInput").ap()
    wkq_ap = nc.dram_tensor("wkq", [C, 128], F16, kind="ExternalInput").ap()
    wkv_ap = nc.dram_tensor("wkv", [C, 128], F16, kind="ExternalInput").ap()
    wv_ap = nc.dram_tensor("wv", [C, H], F16, kind="ExternalInput").ap()
    m0_ap = nc.dram_tensor("m0", [128, 128], F16, kind="ExternalInput").ap()
    m1_ap = nc.dram_tensor("m1", [128, 128], F16, kind="ExternalInput").ap()
    ident_ap = nc.dram_tensor("ident", [64, 64], F16, kind="ExternalInput").ap()
    ones_ap = nc.dram_tensor("ones", [128, 1], F16, kind="ExternalInput").ap()
    outT_ap = nc.dram_tensor("outT", [H + 1, TL], F32, kind="ExternalOutput").ap()

    with tile.TileContext(nc) as tc:
        with (
            tc.tile_pool(name="consts", bufs=1) as consts,
            tc.tile_pool(name="persist", bufs=1) as persist,
            tc.tile_pool(name="xin", bufs=3) as xin,
            tc.tile_pool(name="vt", bufs=2) as vtp,
            tc.tile_pool(name="pb", bufs=3) as pbp,
            tc.tile_pool(name="ob", bufs=2) as obp,
            tc.tile_pool(name="psA", bufs=2, space="PSUM") as psA,
            tc.tile_pool(name="psT", bufs=1, space="PSUM") as psT,
            tc.tile_pool(name="psS", bufs=2, space="PSUM") as psS,
            tc.tile_pool(name="psO", bufs=1, space="PSUM") as psO,
        ):
            # ---- constants ----
            wkq_sb = consts.tile([128, NE, 128], F16)
            wkv_sb = consts.tile([128, NE, 128], F16)
            wv_sb = consts.tile([128, NE, H], F16)
            for dst, src in ((wkq_sb, wkq_ap), (wkv_sb, wkv_ap), (wv_sb, wv_ap)):
                nc.sync.dma_start(
                    out=dst[:], in_=src.rearrange("(c p) h -> p c h", p=128))
            m0_sb = consts.tile([128, 128], F16)
            m1_sb = consts.tile([128, 128], F16)
            ident = consts.tile([64, 64], F16)
            ones_sb = consts.tile([128, 1], F16)
            nc.sync.dma_start(out=m0_sb[:], in_=m0_ap[:])
            nc.sync.dma_start(out=m1_sb[:], in_=m1_ap[:])
            nc.sync.dma_start(out=ident[:], in_=ident_ap[:])
            nc.sync.dma_start(out=ones_sb[:], in_=ones_ap[:])

            # ---- persistent projections (128-partition, bottom half zero) ----
            kT_sb = [persist.tile([128, TL], F16, name=f"kT{p}", tag=f"kT{p}")
                     for p in (0, 1)]
            qT_sb = persist.tile([128, TL], F16)
            v_sb = [persist.tile([128, NB, H + 1], F16, name=f"v{p}", tag=f"v{p}")
                    for p in (0, 1)]
            nc.vector.memset(kT_sb[0][64:128, :], 0.0)
            nc.vector.memset(kT_sb[1][64:128, :], 0.0)
            nc.vector.memset(qT_sb[64:128, :], 0.0)

            # ---- projections ----
            for par, xt_ap in ((0, xt0_ap), (1, xt1_ap)):
                for t in range(NQT):
                    cols = slice(t * 512, (t + 1) * 512)
                    x_sb = xin.tile([128, NE, 512], F16)
                    nc.sync.dma_start(
                        out=x_sb[:],
                        in_=xt_ap[:, cols].rearrange("(c p) m -> p c m", p=128))

                    if par == 0:
                        # K | Q packed
                        kq_ps = psA.tile([128, 512], F32, tag="proj")
                        for e in range(NE):
                            nc.tensor.matmul(kq_ps[:], wkq_sb[:, e, :],
                                             x_sb[:, e, :],
                                             start=(e == 0), stop=(e == NE - 1))
                        nc.vector.tensor_copy(kT_sb[0][0:64, cols],
                                              kq_ps[0:64, :])
                        nc.vector.tensor_copy(qT_sb[0:64, cols],
                                              kq_ps[64:128, :])
                        vt_ps = psA.tile([128, 512], F32, tag="proj")
                        for e in range(NE):
                            nc.tensor.matmul(vt_ps[0:64, :], wv_sb[:, e, :],
                                             x_sb[:, e, :],
                                             start=(e == 0), stop=(e == NE - 1))
                        vT_sb = vtp.tile([64, 512], F16)
                        nc.vector.tensor_copy(vT_sb[:], vt_ps[0:64, :])
                    else:
                        # K | V^T packed
                        kv_ps = psA.tile([128, 512], F32, tag="proj")
                        for e in range(NE):
                            nc.tensor.matmul(kv_ps[:], wkv_sb[:, e, :],
                                             x_sb[:, e, :],
                                             start=(e == 0), stop=(e == NE - 1))
                        nc.vector.tensor_copy(kT_sb[1][0:64, cols],
                                              kv_ps[0:64, :])
                        vT_sb = vtp.tile([64, 512], F16)
                        nc.vector.tensor_copy(vT_sb[:], kv_ps[64:128, :])

                    for u in range(4):
                        tps = psT.tile([128, 64], F16)
                        nc.tensor.transpose(tps[:], vT_sb[:, u * 128:(u + 1) * 128],
                                            ident[:])
                        blk = 4 * t + u
                        nc.vector.tensor_copy(v_sb[par][:, blk, 0:H], tps[:])
                # denominator ones column
                for blk in range(NB):
                    nc.vector.tensor_copy(v_sb[par][:, blk, H:H + 1], ones_sb[:])

            # ---- attention ----
            for qt in range(NQT):
                o_ps = psO.tile([H + 1, 512], F32)
                nkb = 4 * qt + 4
                for s in (0, 1):
                    kT = kT_sb[s]
                    vv = v_sb[s]
                    msk = m0_sb if s == 0 else m1_sb
                    for g in range(0, nkb, 2):
                        kbs = [g, g + 1]
                        offs, widths = [], []
                        pos = 0
                        for kb in kbs:
                            qs = max(0, (kb - 4 * qt) * 128)
                            offs.append((pos, qs))
                            widths.append(512 - qs)
                            pos += 512 - qs
                        sg = psS.tile([128, 1024], F32, tag="scores")
                        for (pos0, qs), w, kb in zip(offs, widths, kbs):
                            nc.tensor.matmul(
                                sg[:, pos0:pos0 + w],
                                kT[:, kb * 128:(kb + 1) * 128],
                                qT_sb[:, qt * 512 + qs:(qt + 1) * 512],
                                start=True, stop=True)
                        p_sb = pbp.tile([128, 1024], F16, tag="probs")
                        nc.scalar.activation(p_sb[:, 0:pos], sg[:, 0:pos], EXP,
                                             scale=float(SCALE))
                        for (pos0, qs), w, kb in zip(offs, widths, kbs):
                            if kb >= 4 * qt:
                                nc.vector.tensor_mul(
                                    p_sb[:, pos0:pos0 + 128],
                                    p_sb[:, pos0:pos0 + 128], msk[:])
                            nc.tensor.matmul(
                                o_ps[:, qs:512],
                                vv[:, kb, :],
                                p_sb[:, pos0:pos0 + w],
                                start=(s == 0 and kb == 0),
                                stop=(s == 1 and kb == nkb - 1))
                o_sb = obp.tile([H + 1, 512], F32)
                nc.vector.tensor_copy(o_sb[:], o_ps[:])
                nc.sync.dma_start(out=outT_ap[:, qt * 512:(qt + 1) * 512],
                                  in_=o_sb[:])

    nc.compile()
    return nc


def _get_program():
    if "nc" not in _CACHE:
        _CACHE["nc"] = _build_program()
    return _CACHE["nc"]


def kernel(x, Wk, Wq, Wv, i, embed_dim, head_size_sel, **_unused):
    from concourse import bass_utils

    x = np.asarray(x, dtype=np.float32)
    Wk = np.asarray(Wk, dtype=np.float32)
    Wq = np.asarray(Wq, dtype=np.float32)
    Wv = np.asarray(Wv, dtype=np.float32)

    nc = _get_program()

    idx = np.arange(128)
    m_incl = (idx[:, None] <= idx[None, :]).astype(np.float16)
    m_strict = (idx[:, None] < idx[None, :]).astype(np.float16)
    ident = np.eye(64, dtype=np.float16)
    ones = np.ones((128, 1), dtype=np.float16)
    wkq = np.concatenate([Wk.T, Wq.T], axis=1).astype(np.float16)  # [C, 128]
    wkv = np.concatenate([Wk.T, Wv.T], axis=1).astype(np.float16)
    wv_t = np.ascontiguousarray(Wv.T).astype(np.float16)           # [C, H]
    xh = x.astype(np.float16)

    in_maps = []
    for c in range(NCORES):
        b, h = c // 2, c % 2
        in_maps.append({
            "xt0": np.ascontiguousarray(xh[b, h::2, :].T),
            "xt1": np.ascontiguousarray(xh[b, 1 - h::2, :].T),
            "wkq": wkq, "wkv": wkv, "wv": wv_t,
            "m0": m_incl,
            "m1": m_strict if h == 0 else m_incl,
            "ident": ident,
            "ones": ones,
        })

    res = bass_utils.run_bass_kernel_spmd(nc, in_maps,
                                          core_ids=list(range(NCORES)))
    _CACHE["last_result"] = res

    out = np.empty((B, T, H), dtype=np.float32)
    for c in range(NCORES):
        b, h = c // 2, c % 2
        outT = res.results[c]["outT"]
        num = outT[:H, :]          # [H, TL]
        den = outT[H, :]           # [TL]
        out[b, h::2, :] = (num / den[None, :]).T
    return out
